# revision 8
# baseline (speedup 1.0000x reference)
"""D-MPNN encoder layer on 8 Trainium2 NeuronCores (Bass/Tile).

Sharding strategy
-----------------
Edge pairs are partitioned across 8 cores (50k pairs -> 100k directed edges
per core); the node space is split into two halves (NH=25088) so every gather
table has < 32768 rows (int16 dma_gather indices). Small weight matrices are
replicated.

Per core, edges are grouped into 4 classes by (src-half, dst-half). Within a
class, edges are stored in dst-window-dense order: 196 windows of 128 nodes,
128 dense slots per window plus 16 overflow slots, interleaved in groups of
32 windows (4096 dense + 512 overflow slots per group). The segment-sum is
fused into the edge phase: each group's h tile is consumed directly from SBUF
by per-window one-hot matmuls accumulating in PSUM, so h is never re-gathered.

For layers >= 1 the source-node and reverse-edge fetches are transpose-mode
dma_gathers (feature-major [128, nslots]), so the W_h matmul consumes the
message tile as lhsT directly -- no PE transposes. Per-half node partials
accumulate in SBUF and are flushed once per window group; each half's
ReduceScatter+AllGather (cheaper than AllReduce here) fires as soon as the
half completes and overlaps the other half / next layer. The initial atom
projection is computed redundantly on every core (no startup collective);
the final layer ReduceScatters each half so core c keeps rows
[c*3136,(c+1)*3136) of both halves; the host reassembles.
"""

import sys
import numpy as np

sys.path.insert(0, "/opt/trn_rl_repo")

# ---------------------------------------------------------------- constants
N_NODES = 50000
N_PAIRS = 400000
ATOM_FDIM = 133
BOND_FDIM = 14
HIDDEN = 128
DEPTH = 3
N_CORES = 8
NH = 25088                      # node half size
NWIN = NH // 128                # 196 windows per half
NH8 = NH // N_CORES             # 3136 rows per core per half
NPAD = 2 * NH

OV = 16                         # overflow slots per (class, window)
CAP = 128 + OV
WGRP = 32                       # windows per group
NGRP = 7                        # 6 full groups + 1 partial (4 windows)
GRP_NW = [WGRP] * 6 + [NWIN - 6 * WGRP]          # [32]*6 + [4]
GRP_SL = [nw * 128 + (nw // 8 if nw % 8 == 0 else (nw + 7) // 8) * 0
          for nw in GRP_NW]     # placeholder, fixed below
GRP_SL = [nw * 128 + ((nw + 7) // 8) * 128 for nw in GRP_NW]  # 4608.., 640
GRP_OFF = np.concatenate([[0], np.cumsum(GRP_SL)]).astype(np.int64)
S_CLS = int(GRP_OFF[-1])        # 28288
S_TOT = 4 * S_CLS

F16 = np.float16
I16 = np.int16

CLS_NAMES = ["00", "01", "10", "11"]
REV_CLS = {"00": "00", "01": "10", "10": "01", "11": "11"}
CLS_OF_D = {0: ["00", "10"], 1: ["01", "11"]}


def _wrap_idx(idx):
    """int16 index array -> dma_gather SBUF layout [128, n/16]."""
    n = idx.shape[0]
    assert n % 16 == 0
    return np.tile(idx.reshape(n // 16, 16).T, (8, 1)).copy()


def _balance_pairs(u, v):
    """Assign pairs to cores so no (core, class, dst-window) exceeds CAP."""
    hu = (u >= NH).astype(np.int64)
    hv = (v >= NH).astype(np.int64)
    cell1 = (hu * 2 + hv) * NWIN + (v - hv * NH) // 128
    cell2 = (hv * 2 + hu) * NWIN + (u - hu * NH) // 128
    assign = np.repeat(np.arange(N_CORES), N_PAIRS // N_CORES)
    loads = np.zeros((N_CORES, 4 * NWIN), np.int64)
    np.add.at(loads, (assign, cell1), 1)
    np.add.at(loads, (assign, cell2), 1)
    rng = np.random.default_rng(0)
    for _ in range(200):
        over = loads > CAP
        if not over.any():
            break
        bad = np.nonzero(over[assign, cell1] | over[assign, cell2])[0]
        rng.shuffle(bad)
        for p in bad:
            c = assign[p]
            if loads[c, cell1[p]] <= CAP and loads[c, cell2[p]] <= CAP:
                continue
            score = np.maximum(loads[:, cell1[p]], loads[:, cell2[p]])
            score[c] += 10000
            cand = int(np.argmin(score))
            if max(loads[cand, cell1[p]], loads[cand, cell2[p]]) + 1 <= CAP:
                loads[c, cell1[p]] -= 1
                loads[c, cell2[p]] -= 1
                loads[cand, cell1[p]] += 1
                loads[cand, cell2[p]] += 1
                assign[p] = cand
    assert (loads <= CAP).all(), "pair balancing failed"
    return assign


def _slot_pos(w, i):
    """Dense slot i of window w -> position within class layout."""
    g = np.minimum(w // WGRP, NGRP - 1)
    return GRP_OFF[g] + (w - g * WGRP) * 128 + i


def _ov_pos(w, k):
    """Overflow slot k (0..15) of window w -> position within class layout."""
    oc = w // 8                       # overflow chunk (8 windows each)
    g = np.minimum(oc // 4, NGRP - 1)
    dense_sz = np.where(g < NGRP - 1, WGRP * 128, GRP_NW[-1] * 128)
    return GRP_OFF[g] + dense_sz + (oc - g * 4) * 128 + (w % 8) * OV + k


def _host_prep(atom_feats, bond_feats, W_i, W_h, W_o, src, dst):
    src = np.asarray(src).astype(np.int64)
    dst = np.asarray(dst).astype(np.int64)
    u = src[:N_PAIRS]
    v = dst[:N_PAIRS]
    assign = _balance_pairs(u, v)

    # directed edges
    eu = np.concatenate([u, v])
    ev = np.concatenate([v, u])
    egid = np.arange(2 * N_PAIRS, dtype=np.int64)
    ecore = np.concatenate([assign, assign])
    hs = (eu >= NH).astype(np.int64)
    hd = (ev >= NH).astype(np.int64)
    ecls = hs * 2 + hd
    dloc = ev - hd * NH
    w = dloc // 128
    dl = dloc % 128
    sloc = eu - hs * NH

    key = ((ecore * 4 + ecls) * NWIN + w)
    order = np.argsort(key, kind="stable")
    key_s = key[order]
    cnt = np.bincount(key_s, minlength=N_CORES * 4 * NWIN)
    starts = np.zeros(N_CORES * 4 * NWIN + 1, dtype=np.int64)
    np.cumsum(cnt, out=starts[1:])
    rank = np.arange(2 * N_PAIRS, dtype=np.int64) - starts[key_s]
    assert rank.max() < CAP, "window capacity exceeded"

    w_s = w[order]
    dense = rank < 128
    pos = np.where(dense, _slot_pos(w_s, np.minimum(rank, 127)),
                   _ov_pos(w_s, np.maximum(rank - 128, 0)))
    abs_pos = ecls[order] * S_CLS + pos                 # within-core position
    core_s = ecore[order]
    gid_s = egid[order]

    # position lookup per edge gid (per core; gids are globally unique)
    pos_of_gid = np.zeros(2 * N_PAIRS, dtype=np.int64)
    pos_of_gid[gid_s] = abs_pos
    rev_gid = np.where(egid < N_PAIRS, egid + N_PAIRS, egid - N_PAIRS)

    bond_feats = np.asarray(bond_feats, dtype=np.float32)
    atom_pad = np.zeros((NPAD, ATOM_FDIM), dtype=np.float32)
    atom_pad[:N_NODES] = np.asarray(atom_feats, dtype=np.float32)
    atomT = np.ascontiguousarray(atom_pad.T).astype(F16)   # [133, NPAD]

    W_i = np.asarray(W_i, dtype=np.float32)
    W_h = np.asarray(W_h, dtype=np.float32)
    W_o = np.asarray(W_o, dtype=np.float32)
    wiaT = np.ascontiguousarray(W_i[:, :ATOM_FDIM].T).astype(F16)
    wibT = np.ascontiguousarray(W_i[:, ATOM_FDIM:].T).astype(F16)
    whT = np.ascontiguousarray(W_h.T).astype(F16)
    woaT = np.ascontiguousarray(W_o[:, :ATOM_FDIM].T).astype(F16)
    womT = np.ascontiguousarray(W_o[:, ATOM_FDIM:].T).astype(F16)

    iotaf = np.tile(np.arange(128, dtype=F16)[None, :], (128, WGRP)).copy()
    ident = np.eye(128, dtype=F16)

    shared = {
        "wia_a": wiaT[:128].copy(), "wia_b": wiaT[128:ATOM_FDIM].copy(),
        "wib": wibT, "wh": whT,
        "woa_a": woaT[:128].copy(), "woa_b": woaT[128:ATOM_FDIM].copy(),
        "wom": womT, "iotaf": iotaf, "ident": ident,
        "atomT_a": atomT[:128].copy(), "atomT_b": atomT[128:ATOM_FDIM].copy(),
    }

    in_maps = []
    for c in range(N_CORES):
        m = np.nonzero(core_s == c)[0]
        ap = abs_pos[m]
        src16 = np.zeros(S_TOT, dtype=I16)
        rev16 = np.zeros(S_TOT, dtype=I16)
        gids = gid_s[m]
        src16[ap] = sloc[order][m].astype(I16)
        rev16[ap] = (pos_of_gid[rev_gid[gids]] % S_CLS).astype(I16)
        bondT = np.zeros((BOND_FDIM, S_TOT), dtype=F16)
        bondT[:, ap] = bond_feats[gids].T.astype(F16)

        # dstl: per class, dense cols then overflow cols, [128, 4*2*NWIN]
        dstl = np.full((128, 4 * 2 * NWIN), 200.0, dtype=np.float32)
        cls_m = ecls[order][m]
        w_m = w_s[m]
        dl_m = dl[order][m]
        rank_m = rank[m]
        dn = rank_m < 128
        ci = cls_m[dn]
        dstl[rank_m[dn], ci * 2 * NWIN + w_m[dn]] = dl_m[dn]
        ovm = ~dn
        ci = cls_m[ovm]
        row = (w_m[ovm] % 8) * OV + (rank_m[ovm] - 128)
        dstl[row, ci * 2 * NWIN + NWIN + w_m[ovm]] = dl_m[ovm]

        # per-core atom columns for the final stage: owned rows of each half
        s0 = slice(c * NH8, (c + 1) * NH8)
        s1 = slice(NH + c * NH8, NH + (c + 1) * NH8)
        atomP_a = np.concatenate([atomT[:128, s0], atomT[:128, s1]], axis=1)
        atomP_b = np.concatenate([atomT[128:ATOM_FDIM, s0],
                                  atomT[128:ATOM_FDIM, s1]], axis=1)

        d = dict(shared)
        d["bondT"] = bondT
        d["src16w"] = _wrap_idx(src16)
        d["rev16w"] = _wrap_idx(rev16)
        d["dstl"] = dstl.astype(F16)
        d["atomP_a"] = np.ascontiguousarray(atomP_a)
        d["atomP_b"] = np.ascontiguousarray(atomP_b)
        in_maps.append(d)

    meta = dict(S_CLS=S_CLS)
    return meta, in_maps


# ------------------------------------------------------------------ program
def _build_program(meta, debug_1core=False, dbg_dump=False, use_ar=True):
    import concourse.bacc as bacc
    import concourse.tile as tile
    import concourse.mybir as mybir
    from concourse import library_config

    f16, f32, i16 = mybir.dt.float16, mybir.dt.float32, mybir.dt.int16
    Relu = mybir.ActivationFunctionType.Relu
    OUT_COLS = 2 * NH8

    nc = bacc.Bacc("TRN2", target_bir_lowering=False, debug=False,
                   enable_asserts=False,
                   num_devices=1 if debug_1core else N_CORES,
                   num_swdge_queues=4)

    def din(name, shape, dt=f16):
        return nc.dram_tensor(name, shape, dt, kind="ExternalInput").ap()

    atomT_a = din("atomT_a", [128, NPAD])
    atomT_b = din("atomT_b", [ATOM_FDIM - 128, NPAD])
    atomP_a = din("atomP_a", [128, OUT_COLS])
    atomP_b = din("atomP_b", [ATOM_FDIM - 128, OUT_COLS])
    wia_a = din("wia_a", [128, HIDDEN])
    wia_b = din("wia_b", [ATOM_FDIM - 128, HIDDEN])
    wib = din("wib", [BOND_FDIM, HIDDEN])
    wh_t = din("wh", [HIDDEN, HIDDEN])
    woa_a = din("woa_a", [128, HIDDEN])
    woa_b = din("woa_b", [ATOM_FDIM - 128, HIDDEN])
    wom = din("wom", [HIDDEN, HIDDEN])
    iotaf = din("iotaf", [128, WGRP * 128])
    ident_t = din("ident", [128, 128])
    bondT = din("bondT", [BOND_FDIM, S_TOT])
    src16w = din("src16w", [128, S_TOT // 16], i16)
    rev16w = din("rev16w", [128, S_TOT // 16], i16)
    dstl_t = din("dstl", [128, 4 * 2 * NWIN])

    out_t = nc.dram_tensor("out", [HIDDEN, OUT_COLS], f32,
                           kind="ExternalOutput").ap()
    dbg = {}
    if dbg_dump:
        for nm, shp in (("tmp1_0", [NH, HIDDEN]), ("par1_0", [NH, HIDDEN]),
                        ("par2_0", [NH, HIDDEN]), ("rso_0", [NH8, HIDDEN]),
                        ("h1_00", [S_CLS, HIDDEN])):
            dbg[nm] = nc.dram_tensor("dbg_" + nm, shp, f16,
                                     kind="ExternalOutput").ap()

    proj_d = [nc.dram_tensor(f"proj{d}", [NH, HIDDEN], f16,
                             kind="Internal").ap() for d in range(2)]
    h_cls = {}
    for ell in range(DEPTH - 1):
        for cn in CLS_NAMES:
            h_cls[(ell, cn)] = nc.dram_tensor(
                f"h{ell}_{cn}", [S_CLS, HIDDEN], f16, kind="Internal").ap()
    partials = [[nc.dram_tensor(f"partials{ell}_{d}", [NH, HIDDEN], f16,
                                kind="Internal").ap() for d in range(2)]
                for ell in range(DEPTH)]
    rs_mid = [[nc.dram_tensor(f"rsmid{ell}_{d}", [NH8, HIDDEN], f16,
                              kind="Internal").ap() for d in range(2)]
              for ell in range(DEPTH - 1)]
    tmp = [[nc.dram_tensor(f"tmp{ell}_{d}", [NH, HIDDEN], f16,
                           kind="Internal", addr_space="Shared").ap()
            for d in range(2)] for ell in range(DEPTH - 1)]
    rs_out = [nc.dram_tensor(f"rsout{d}", [NH8, HIDDEN], f16,
                             kind="Internal").ap() for d in range(2)]

    nc.gpsimd.load_library(library_config.mlp)

    with tile.TileContext(nc) as tc:
        with (
            tc.tile_pool(name="pers", bufs=1) as pers,
            tc.tile_pool(name="work", bufs=2) as work,
            tc.tile_pool(name="selp", bufs=1) as selp,
            tc.tile_pool(name="parp", bufs=1) as parp,
            tc.tile_pool(name="pse", bufs=2, space="PSUM") as pse,
            tc.tile_pool(name="psA", bufs=2, space="PSUM") as psA,
            tc.tile_pool(name="psT", bufs=4, space="PSUM") as psT,
        ):
            # ---------- persistent SBUF
            def pload(ap_in, shape, tag, dt=f16, eng="sync"):
                t = pers.tile(shape, dt, tag=tag)
                (nc.sync if eng == "sync" else nc.scalar).dma_start(t[:], ap_in)
                return t

            w_wia_a = pload(wia_a[:], [128, HIDDEN], "w_wia_a")
            w_wia_b = pload(wia_b[:], [ATOM_FDIM - 128, HIDDEN], "w_wia_b")
            w_wib = pload(wib[:], [BOND_FDIM, HIDDEN], "w_wib")
            w_wh = pload(wh_t[:], [HIDDEN, HIDDEN], "w_wh")
            w_woa_a = pload(woa_a[:], [128, HIDDEN], "w_woa_a")
            w_woa_b = pload(woa_b[:], [ATOM_FDIM - 128, HIDDEN], "w_woa_b")
            w_wom = pload(wom[:], [HIDDEN, HIDDEN], "w_wom")
            io_t = pload(iotaf[:], [128, WGRP * 128], "io_t")
            ident_s = pload(ident_t[:], [128, 128], "ident_s")
            sidx = pload(src16w[:], [128, S_TOT // 16], "sidx", i16)
            ridx = pload(rev16w[:], [128, S_TOT // 16], "ridx", i16,
                         eng="scalar")
            dstl_s = pload(dstl_t[:], [128, 4 * 2 * NWIN], "dstl_s",
                           eng="scalar")

            # ---------- redundant proj: full node table on every core
            # NH = 49*512, so 512-col chunks align to halves.
            for half in range(2):
                for ch in range(NH // 512):
                    base = half * NH + ch * 512
                    a_t = work.tile([128, 512], f16, tag="pa")
                    b_t = work.tile([ATOM_FDIM - 128, 512], f16, tag="pb")
                    nc.sync.dma_start(a_t[:], atomT_a[:, base:base + 512])
                    nc.scalar.dma_start(b_t[:], atomT_b[:, base:base + 512])
                    o_t = work.tile([128, 512], f16, tag="po")
                    for q in range(4):
                        ps = psA.tile([128, HIDDEN], f32, tag="sg")
                        qs = slice(q * 128, (q + 1) * 128)
                        nc.tensor.matmul(ps[:], lhsT=a_t[:, qs],
                                         rhs=w_wia_a[:], start=True,
                                         stop=False)
                        nc.tensor.matmul(ps[:], lhsT=b_t[:, qs],
                                         rhs=w_wia_b[:], start=False,
                                         stop=True)
                        nc.scalar.copy(o_t[:, qs], ps[:])
                    nc.sync.dma_start(
                        proj_d[half][ch * 512:(ch + 1) * 512, :].rearrange(
                            "(a p) d -> p a d", p=128),
                        o_t[:].rearrange("p (a d) -> p a d", d=HIDDEN))

            # ---------- layers with fused segment-sum
            for ell in range(DEPTH):
                for d in (0, 1):
                    par_t = parp.tile([128, NWIN * HIDDEN], f16, tag="par")
                    for ci_pos, cn in enumerate(CLS_OF_D[d]):
                        s = int(cn[0])
                        cls_idx = CLS_NAMES.index(cn)
                        o = cls_idx * S_CLS
                        rcn = REV_CLS[cn]
                        for g in range(NGRP):
                            nw = GRP_NW[g]
                            nsl = GRP_SL[g]
                            goff = int(GRP_OFF[g])
                            nov = ((nw + 7) // 8) * 128
                            w0 = g * WGRP
                            icols = slice((o + goff) // 16,
                                          (o + goff + nsl) // 16)
                            h_t = work.tile([128, GRP_SL[0]], f16, tag="ht")
                            if ell == 0:
                                g1 = work.tile([128, GRP_SL[0]], f16,
                                               tag="g1")
                                nc.gpsimd.dma_gather(
                                    g1[:, :nsl].rearrange(
                                        "p (c n) -> p c n", n=HIDDEN),
                                    proj_d[s][:], sidx[:, icols], nsl, nsl,
                                    HIDDEN, single_packet=False,
                                    queue_num=(2 * g + ci_pos) % 4)
                                bt = work.tile([BOND_FDIM, GRP_SL[0]], f16,
                                               tag="bt")
                                nc.scalar.dma_start(
                                    bt[:, :nsl],
                                    bondT[:, o + goff:o + goff + nsl])
                                for j in range((nsl + 511) // 512):
                                    nq = min(4, nsl // 128 - j * 4)
                                    ps = pse.tile([128, 512], f32, tag="mm")
                                    for q in range(nq):
                                        ci = j * 4 + q
                                        nc.tensor.matmul(
                                            ps[:, q * 128:(q + 1) * 128],
                                            lhsT=bt[:, ci * 128:
                                                    (ci + 1) * 128],
                                            rhs=w_wib[:], start=True,
                                            stop=True)
                                    sl = slice(j * 512, j * 512 + nq * 128)
                                    nc.vector.tensor_add(
                                        out=h_t[:, sl], in0=g1[:, sl],
                                        in1=ps[:, :nq * 128])
                                    nc.scalar.activation(h_t[:, sl],
                                                         h_t[:, sl], Relu)
                            else:
                                table = tmp[ell - 1][s]
                                g1 = work.tile([128, GRP_SL[0]], f16,
                                               tag="g1")
                                nc.gpsimd.dma_gather(
                                    g1[:, :nsl].rearrange(
                                        "p (c d) -> p c d", d=HIDDEN),
                                    table[:], sidx[:, icols], nsl, nsl,
                                    HIDDEN, single_packet=False,
                                    queue_num=(2 * g + ci_pos) % 4)
                                g2 = work.tile([128, GRP_SL[0]], f16,
                                               tag="g2")
                                nc.gpsimd.dma_gather(
                                    g2[:, :nsl].rearrange(
                                        "p (c d) -> p c d", d=HIDDEN),
                                    h_cls[(ell - 1, rcn)][:],
                                    ridx[:, icols], nsl, nsl,
                                    HIDDEN, single_packet=False,
                                    queue_num=(2 * g + ci_pos + 2) % 4)
                                nc.vector.tensor_tensor(
                                    out=g1[:, :nsl], in0=g1[:, :nsl],
                                    in1=g2[:, :nsl],
                                    op=mybir.AluOpType.subtract)
                                for j in range((nsl + 511) // 512):
                                    nq = min(4, nsl // 128 - j * 4)
                                    ps = pse.tile([128, 512], f32, tag="mm")
                                    mt = work.tile([128, 512], f16, tag="mt")
                                    for q in range(nq):
                                        ci = j * 4 + q
                                        tp = psT.tile([128, 128], f16,
                                                      tag="tp")
                                        nc.tensor.transpose(
                                            tp[:],
                                            g1[:, ci * 128:(ci + 1) * 128],
                                            ident_s[:])
                                        msl = slice(q * 128, (q + 1) * 128)
                                        if q % 2 == 0:
                                            nc.scalar.copy(mt[:, msl], tp[:])
                                        else:
                                            nc.vector.tensor_copy(
                                                mt[:, msl], tp[:])
                                        nc.tensor.matmul(
                                            ps[:, msl], lhsT=mt[:, msl],
                                            rhs=w_wh[:], start=True,
                                            stop=True)
                                    sl = slice(j * 512, j * 512 + nq * 128)
                                    nc.scalar.activation(
                                        h_t[:, sl], ps[:, :nq * 128], Relu)
                            if ell < DEPTH - 1:
                                nc.sync.dma_start(
                                    h_cls[(ell, cn)]
                                    [goff:goff + nsl, :].rearrange(
                                        "(c p) d -> p c d", p=128),
                                    h_t[:, :nsl].rearrange(
                                        "p (c d) -> p c d", d=HIDDEN))

                            # fused seg-sum for this group's windows
                            sel_d = selp.tile([128, WGRP * 128], f16,
                                              tag="seld")
                            sel_o = selp.tile([128, WGRP * 128], f16,
                                              tag="selo")
                            dc = cls_idx * 2 * NWIN + w0
                            nc.vector.tensor_tensor(
                                out=sel_d[:, :nw * 128].rearrange(
                                    "p (c n) -> p c n", n=128),
                                in0=io_t[:, :nw * 128].rearrange(
                                    "p (c n) -> p c n", n=128),
                                in1=dstl_s[:, dc:dc + nw]
                                .to_broadcast([128, nw, 128]),
                                op=mybir.AluOpType.is_equal)
                            dco = cls_idx * 2 * NWIN + NWIN + w0
                            nc.vector.tensor_tensor(
                                out=sel_o[:, :nw * 128].rearrange(
                                    "p (c n) -> p c n", n=128),
                                in0=io_t[:, :nw * 128].rearrange(
                                    "p (c n) -> p c n", n=128),
                                in1=dstl_s[:, dco:dco + nw]
                                .to_broadcast([128, nw, 128]),
                                op=mybir.AluOpType.is_equal)
                            for wi in range(nw):
                                ps2 = psA.tile([128, HIDDEN], f32, tag="sg")
                                nc.tensor.matmul(
                                    ps2[:],
                                    lhsT=sel_d[:, wi * 128:(wi + 1) * 128],
                                    rhs=h_t[:, wi * HIDDEN:
                                            (wi + 1) * HIDDEN],
                                    start=True, stop=False)
                                oc = nw + wi // 8
                                nc.tensor.matmul(
                                    ps2[:],
                                    lhsT=sel_o[:, wi * 128:(wi + 1) * 128],
                                    rhs=h_t[:, oc * HIDDEN:
                                            (oc + 1) * HIDDEN],
                                    start=False, stop=True)
                                col = (w0 + wi) * HIDDEN
                                if ci_pos == 0:
                                    nc.scalar.copy(
                                        par_t[:, col:col + HIDDEN], ps2[:])
                                else:
                                    nc.vector.tensor_add(
                                        out=par_t[:, col:col + HIDDEN],
                                        in0=par_t[:, col:col + HIDDEN],
                                        in1=ps2[:])
                            if ci_pos == 1:
                                nc.sync.dma_start(
                                    partials[ell][d]
                                    [w0 * 128:w0 * 128 + nw * 128, :]
                                    .rearrange("(a p) d -> p a d", p=128),
                                    par_t[:, w0 * HIDDEN:
                                          (w0 + nw) * HIDDEN].rearrange(
                                        "p (a d) -> p a d", d=HIDDEN))
                    # collectives per half
                    if debug_1core:
                        if ell < DEPTH - 1:
                            nc.sync.dma_start(tmp[ell][d][:],
                                              partials[ell][d][:])
                        else:
                            nc.sync.dma_start(rs_out[d][:],
                                              partials[ell][d][0:NH8, :])
                    elif ell < DEPTH - 1:
                        if use_ar:
                            nc.gpsimd.collective_compute(
                                "AllReduce", mybir.AluOpType.add,
                                replica_groups=[list(range(N_CORES))],
                                ins=[partials[ell][d][:]],
                                outs=[tmp[ell][d][:]])
                        else:
                            nc.gpsimd.collective_compute(
                                "ReduceScatter", mybir.AluOpType.add,
                                replica_groups=[list(range(N_CORES))],
                                ins=[partials[ell][d][:]],
                                outs=[rs_mid[ell][d][:]])
                            nc.gpsimd.collective_compute(
                                "AllGather", mybir.AluOpType.bypass,
                                replica_groups=[list(range(N_CORES))],
                                ins=[rs_mid[ell][d][:]],
                                outs=[tmp[ell][d][:]])
                    else:
                        nc.gpsimd.collective_compute(
                            "ReduceScatter", mybir.AluOpType.add,
                            replica_groups=[list(range(N_CORES))],
                            ins=[partials[ell][d][:]], outs=[rs_out[d][:]])

            if dbg_dump:
                nc.sync.dma_start(dbg["tmp1_0"][:], tmp[1][0][:])
                nc.scalar.dma_start(dbg["par1_0"][:], partials[1][0][:])
                nc.sync.dma_start(dbg["par2_0"][:], partials[2][0][:])
                nc.scalar.dma_start(dbg["rso_0"][:], rs_out[0][:])
                nc.sync.dma_start(dbg["h1_00"][:], h_cls[(1, "00")][:])

            # ---------- final: out.T = relu(WoA@atom.T + WoM@msg.T)
            for ch in range(OUT_COLS // 128):
                csl = slice(ch * 128, (ch + 1) * 128)
                a_t = work.tile([128, 128], f16, tag="fa")
                b_t = work.tile([ATOM_FDIM - 128, 128], f16, tag="fb")
                m_t = work.tile([128, 128], f16, tag="fm")
                mraw = work.tile([128, 128], f16, tag="fmr")
                nc.sync.dma_start(a_t[:], atomP_a[:, csl])
                nc.scalar.dma_start(b_t[:], atomP_b[:, csl])
                r0 = ch * 128
                r1 = (ch + 1) * 128
                if r1 <= NH8:
                    nc.sync.dma_start(mraw[:], rs_out[0][r0:r1, :])
                elif r0 >= NH8:
                    nc.sync.dma_start(mraw[:], rs_out[1][r0 - NH8:r1 - NH8, :])
                else:
                    k = NH8 - r0
                    nc.sync.dma_start(mraw[:k, :], rs_out[0][r0:NH8, :])
                    nc.sync.dma_start(mraw[k:, :], rs_out[1][0:r1 - NH8, :])
                tpf = psT.tile([128, 128], f16, tag="tp")
                nc.tensor.transpose(tpf[:], mraw[:], ident_s[:])
                nc.scalar.copy(m_t[:], tpf[:])
                ps = psA.tile([128, 128], f32, tag="sg")
                nc.tensor.matmul(ps[:], lhsT=w_woa_a[:], rhs=a_t[:],
                                 start=True, stop=False)
                nc.tensor.matmul(ps[:], lhsT=w_woa_b[:], rhs=b_t[:],
                                 start=False, stop=False)
                nc.tensor.matmul(ps[:], lhsT=w_wom[:], rhs=m_t[:],
                                 start=False, stop=True)
                o_t = work.tile([128, 128], f32, tag="fob")
                nc.scalar.activation(o_t[:], ps[:], Relu)
                nc.sync.dma_start(out_t[:, csl], o_t[:])

    # Tile assigns SWDGE completion sems round-robin (DMASW<i>_*); the HW
    # locks each sem to one SWDGE queue, so spread gathers across the 4
    # queues by their assigned sem index.
    import re
    for b in nc.main_func.blocks:
        for ins in b.instructions:
            if type(ins).__name__ == "InstDMAGatherAnt" and ins.sync_info:
                for upd in ins.sync_info.on_update:
                    mname = upd.ant_name or ""
                    mm = re.match(r"DMASW(\d+)_", mname)
                    if mm:
                        ins.queue_num = int(mm.group(1)) % 4
                        break

    nc.compile()
    return nc


# -------------------------------------------------------------------- entry
_CACHE = {}


def kernel(atom_feats, bond_feats, W_i, W_h, W_o, src, dst, reverse_e):
    from concourse import bass_utils

    rev = np.asarray(reverse_e).astype(np.int64)
    ar = np.arange(N_PAIRS, dtype=np.int64)
    assert np.array_equal(rev[:N_PAIRS], ar + N_PAIRS) and \
        np.array_equal(rev[N_PAIRS:], ar), "unexpected reverse_e structure"

    meta, in_maps = _host_prep(atom_feats, bond_feats, W_i, W_h, W_o, src, dst)

    key = (meta["S_CLS"],)
    if key not in _CACHE:
        _CACHE[key] = _build_program(meta)
    nc = _CACHE[key]

    res = bass_utils.run_bass_kernel_spmd(
        nc, in_maps, core_ids=list(range(N_CORES)))
    out = np.empty((NPAD, HIDDEN), dtype=np.float32)
    for c in range(N_CORES):
        o = res.results[c]["out"].T.astype(np.float32)   # [OUT_COLS, H]
        out[c * NH8:(c + 1) * NH8] = o[:NH8]
        out[NH + c * NH8:NH + (c + 1) * NH8] = o[NH8:]
    return np.ascontiguousarray(out[:N_NODES])


# revision 11
# speedup vs baseline: 1.5593x; 1.5593x over previous
"""D-MPNN encoder layer on 8 Trainium2 NeuronCores (Bass/Tile).

Sharding strategy
-----------------
Edge pairs are partitioned across 8 cores (50k pairs -> 100k directed edges
per core); the node space is split into two halves (NH=25088) so every gather
table has < 32768 rows (int16 dma_gather indices). Small weight matrices are
replicated.

Per core, edges are grouped into 4 classes by (src-half, dst-half). Within a
class, edges are stored in dst-window-dense order: 196 windows of 128 nodes,
128 dense slots per window plus 16 overflow slots, interleaved in groups of
32 windows (4096 dense + 512 overflow slots per group). The segment-sum is
fused into the edge phase: each group's h tile is consumed directly from SBUF
by per-window one-hot matmuls accumulating in PSUM, so h is never re-gathered.

For layers >= 1 the source-node and reverse-edge fetches are transpose-mode
dma_gathers (feature-major [128, nslots]), so the W_h matmul consumes the
message tile as lhsT directly -- no PE transposes. Per-half node partials
accumulate in SBUF and are flushed once per window group; each half's
ReduceScatter+AllGather (cheaper than AllReduce here) fires as soon as the
half completes and overlaps the other half / next layer. The initial atom
projection is computed redundantly on every core (no startup collective);
the final layer ReduceScatters each half so core c keeps rows
[c*3136,(c+1)*3136) of both halves; the host reassembles.
"""

import sys
import numpy as np

sys.path.insert(0, "/opt/trn_rl_repo")

# ---------------------------------------------------------------- constants
N_NODES = 50000
N_PAIRS = 400000
ATOM_FDIM = 133
BOND_FDIM = 14
HIDDEN = 128
DEPTH = 3
N_CORES = 8
NH = 25088                      # node half size
NWIN = NH // 128                # 196 windows per half
NH8 = NH // N_CORES             # 3136 rows per core per half
NPAD = 2 * NH

OV = 16                         # overflow slots per (class, window)
CAP = 128 + OV
WGRP = 32                       # windows per group
NGRP = 7                        # 6 full groups + 1 partial (4 windows)
GRP_NW = [WGRP] * 6 + [NWIN - 6 * WGRP]          # [32]*6 + [4]
GRP_SL = [nw * 128 + (nw // 8 if nw % 8 == 0 else (nw + 7) // 8) * 0
          for nw in GRP_NW]     # placeholder, fixed below
GRP_SL = [nw * 128 + ((nw + 7) // 8) * 128 for nw in GRP_NW]  # 4608.., 640
GRP_OFF = np.concatenate([[0], np.cumsum(GRP_SL)]).astype(np.int64)
S_CLS = int(GRP_OFF[-1])        # 28288
S_TOT = 4 * S_CLS

F16 = np.float16
I16 = np.int16

CLS_NAMES = ["00", "01", "10", "11"]
REV_CLS = {"00": "00", "01": "10", "10": "01", "11": "11"}
CLS_OF_D = {0: ["00", "10"], 1: ["01", "11"]}


def _wrap_idx(idx):
    """int16 index array -> dma_gather SBUF layout [128, n/16]."""
    n = idx.shape[0]
    assert n % 16 == 0
    return np.tile(idx.reshape(n // 16, 16).T, (8, 1)).copy()


def _balance_pairs(u, v):
    """Assign pairs to cores so no (core, class, dst-window) exceeds CAP."""
    hu = (u >= NH).astype(np.int64)
    hv = (v >= NH).astype(np.int64)
    cell1 = (hu * 2 + hv) * NWIN + (v - hv * NH) // 128
    cell2 = (hv * 2 + hu) * NWIN + (u - hu * NH) // 128
    assign = np.repeat(np.arange(N_CORES), N_PAIRS // N_CORES)
    loads = np.zeros((N_CORES, 4 * NWIN), np.int64)
    np.add.at(loads, (assign, cell1), 1)
    np.add.at(loads, (assign, cell2), 1)
    rng = np.random.default_rng(0)
    for _ in range(200):
        over = loads > CAP
        if not over.any():
            break
        bad = np.nonzero(over[assign, cell1] | over[assign, cell2])[0]
        rng.shuffle(bad)
        for p in bad:
            c = assign[p]
            if loads[c, cell1[p]] <= CAP and loads[c, cell2[p]] <= CAP:
                continue
            score = np.maximum(loads[:, cell1[p]], loads[:, cell2[p]])
            score[c] += 10000
            cand = int(np.argmin(score))
            if max(loads[cand, cell1[p]], loads[cand, cell2[p]]) + 1 <= CAP:
                loads[c, cell1[p]] -= 1
                loads[c, cell2[p]] -= 1
                loads[cand, cell1[p]] += 1
                loads[cand, cell2[p]] += 1
                assign[p] = cand
    assert (loads <= CAP).all(), "pair balancing failed"
    return assign


def _slot_pos(w, i):
    """Dense slot i of window w -> position within class layout."""
    g = np.minimum(w // WGRP, NGRP - 1)
    return GRP_OFF[g] + (w - g * WGRP) * 128 + i


def _ov_pos(w, k):
    """Overflow slot k (0..15) of window w -> position within class layout."""
    oc = w // 8                       # overflow chunk (8 windows each)
    g = np.minimum(oc // 4, NGRP - 1)
    dense_sz = np.where(g < NGRP - 1, WGRP * 128, GRP_NW[-1] * 128)
    return GRP_OFF[g] + dense_sz + (oc - g * 4) * 128 + (w % 8) * OV + k


def _host_prep(atom_feats, bond_feats, W_i, W_h, W_o, src, dst):
    src = np.asarray(src).astype(np.int64)
    dst = np.asarray(dst).astype(np.int64)
    u = src[:N_PAIRS]
    v = dst[:N_PAIRS]
    assign = _balance_pairs(u, v)

    # directed edges
    eu = np.concatenate([u, v])
    ev = np.concatenate([v, u])
    egid = np.arange(2 * N_PAIRS, dtype=np.int64)
    ecore = np.concatenate([assign, assign])
    hs = (eu >= NH).astype(np.int64)
    hd = (ev >= NH).astype(np.int64)
    ecls = hs * 2 + hd
    dloc = ev - hd * NH
    w = dloc // 128
    dl = dloc % 128
    sloc = eu - hs * NH

    key = ((ecore * 4 + ecls) * NWIN + w)
    order = np.argsort(key, kind="stable")
    key_s = key[order]
    cnt = np.bincount(key_s, minlength=N_CORES * 4 * NWIN)
    starts = np.zeros(N_CORES * 4 * NWIN + 1, dtype=np.int64)
    np.cumsum(cnt, out=starts[1:])
    rank = np.arange(2 * N_PAIRS, dtype=np.int64) - starts[key_s]
    assert rank.max() < CAP, "window capacity exceeded"

    w_s = w[order]
    dense = rank < 128
    pos = np.where(dense, _slot_pos(w_s, np.minimum(rank, 127)),
                   _ov_pos(w_s, np.maximum(rank - 128, 0)))
    abs_pos = ecls[order] * S_CLS + pos                 # within-core position
    core_s = ecore[order]
    gid_s = egid[order]

    # position lookup per edge gid (per core; gids are globally unique)
    pos_of_gid = np.zeros(2 * N_PAIRS, dtype=np.int64)
    pos_of_gid[gid_s] = abs_pos
    rev_gid = np.where(egid < N_PAIRS, egid + N_PAIRS, egid - N_PAIRS)

    bond_feats = np.asarray(bond_feats, dtype=np.float32)
    atom_pad = np.zeros((NPAD, ATOM_FDIM), dtype=np.float32)
    atom_pad[:N_NODES] = np.asarray(atom_feats, dtype=np.float32)
    atomT = np.ascontiguousarray(atom_pad.T).astype(F16)   # [133, NPAD]

    W_i = np.asarray(W_i, dtype=np.float32)
    W_h = np.asarray(W_h, dtype=np.float32)
    W_o = np.asarray(W_o, dtype=np.float32)
    wiaT = np.ascontiguousarray(W_i[:, :ATOM_FDIM].T).astype(F16)
    wibT = np.ascontiguousarray(W_i[:, ATOM_FDIM:].T).astype(F16)
    whT = np.ascontiguousarray(W_h.T).astype(F16)
    woaT = np.ascontiguousarray(W_o[:, :ATOM_FDIM].T).astype(F16)
    womT = np.ascontiguousarray(W_o[:, ATOM_FDIM:].T).astype(F16)

    iotaf = np.tile(np.arange(128, dtype=F16)[None, :], (128, WGRP)).copy()
    ident = np.eye(128, dtype=F16)

    shared = {
        "wia_a": wiaT[:128].copy(), "wia_b": wiaT[128:ATOM_FDIM].copy(),
        "wib": wibT, "wh": whT,
        "woa_a": woaT[:128].copy(), "woa_b": woaT[128:ATOM_FDIM].copy(),
        "wom": womT, "iotaf": iotaf, "ident": ident,
        "atomT_a": atomT[:128].copy(), "atomT_b": atomT[128:ATOM_FDIM].copy(),
    }

    in_maps = []
    for c in range(N_CORES):
        m = np.nonzero(core_s == c)[0]
        ap = abs_pos[m]
        src16 = np.zeros(S_TOT, dtype=I16)
        rev16 = np.zeros(S_TOT, dtype=I16)
        gids = gid_s[m]
        src16[ap] = sloc[order][m].astype(I16)
        rev16[ap] = (pos_of_gid[rev_gid[gids]] % S_CLS).astype(I16)
        bondT = np.zeros((BOND_FDIM, S_TOT), dtype=F16)
        bondT[:, ap] = bond_feats[gids].T.astype(F16)

        # dstl: per class, dense cols then overflow cols, [128, 4*2*NWIN]
        dstl = np.full((128, 4 * 2 * NWIN), 200.0, dtype=np.float32)
        cls_m = ecls[order][m]
        w_m = w_s[m]
        dl_m = dl[order][m]
        rank_m = rank[m]
        dn = rank_m < 128
        ci = cls_m[dn]
        dstl[rank_m[dn], ci * 2 * NWIN + w_m[dn]] = dl_m[dn]
        ovm = ~dn
        ci = cls_m[ovm]
        row = (w_m[ovm] % 8) * OV + (rank_m[ovm] - 128)
        dstl[row, ci * 2 * NWIN + NWIN + w_m[ovm]] = dl_m[ovm]

        # per-core atom columns for the final stage: owned rows of each half
        s0 = slice(c * NH8, (c + 1) * NH8)
        s1 = slice(NH + c * NH8, NH + (c + 1) * NH8)
        atomP_a = np.concatenate([atomT[:128, s0], atomT[:128, s1]], axis=1)
        atomP_b = np.concatenate([atomT[128:ATOM_FDIM, s0],
                                  atomT[128:ATOM_FDIM, s1]], axis=1)

        d = dict(shared)
        d["bondT"] = bondT
        d["src16w"] = _wrap_idx(src16)
        d["rev16w"] = _wrap_idx(rev16)
        d["dstl"] = dstl.astype(F16)
        d["atomP_a"] = np.ascontiguousarray(atomP_a)
        d["atomP_b"] = np.ascontiguousarray(atomP_b)
        in_maps.append(d)

    meta = dict(S_CLS=S_CLS)
    return meta, in_maps


# ------------------------------------------------------------------ program
def _build_program(meta, debug_1core=False, dbg_dump=False, use_ar=False):
    import concourse.bacc as bacc
    import concourse.tile as tile
    import concourse.mybir as mybir
    from concourse import library_config

    f16, f32, i16 = mybir.dt.float16, mybir.dt.float32, mybir.dt.int16
    Relu = mybir.ActivationFunctionType.Relu
    OUT_COLS = 2 * NH8

    nc = bacc.Bacc("TRN2", target_bir_lowering=False, debug=False,
                   enable_asserts=False,
                   num_devices=1 if debug_1core else N_CORES,
                   num_swdge_queues=4)

    def din(name, shape, dt=f16):
        return nc.dram_tensor(name, shape, dt, kind="ExternalInput").ap()

    atomT_a = din("atomT_a", [128, NPAD])
    atomT_b = din("atomT_b", [ATOM_FDIM - 128, NPAD])
    atomP_a = din("atomP_a", [128, OUT_COLS])
    atomP_b = din("atomP_b", [ATOM_FDIM - 128, OUT_COLS])
    wia_a = din("wia_a", [128, HIDDEN])
    wia_b = din("wia_b", [ATOM_FDIM - 128, HIDDEN])
    wib = din("wib", [BOND_FDIM, HIDDEN])
    wh_t = din("wh", [HIDDEN, HIDDEN])
    woa_a = din("woa_a", [128, HIDDEN])
    woa_b = din("woa_b", [ATOM_FDIM - 128, HIDDEN])
    wom = din("wom", [HIDDEN, HIDDEN])
    iotaf = din("iotaf", [128, WGRP * 128])
    ident_t = din("ident", [128, 128])
    bondT = din("bondT", [BOND_FDIM, S_TOT])
    src16w = din("src16w", [128, S_TOT // 16], i16)
    rev16w = din("rev16w", [128, S_TOT // 16], i16)
    dstl_t = din("dstl", [128, 4 * 2 * NWIN])

    out_t = nc.dram_tensor("out", [HIDDEN, OUT_COLS], f32,
                           kind="ExternalOutput").ap()
    dbg = {}
    if dbg_dump:
        for nm, shp in (("tmp1_0", [NH, HIDDEN]), ("par1_0", [NH, HIDDEN]),
                        ("par2_0", [NH, HIDDEN]), ("rso_0", [NH8, HIDDEN]),
                        ("h1_00", [S_CLS, HIDDEN])):
            dbg[nm] = nc.dram_tensor("dbg_" + nm, shp, f16,
                                     kind="ExternalOutput").ap()

    proj_d = [nc.dram_tensor(f"proj{d}", [NH, HIDDEN], f16,
                             kind="Internal").ap() for d in range(2)]
    h_cls = {}
    for ell in range(DEPTH - 1):
        for cn in CLS_NAMES:
            h_cls[(ell, cn)] = nc.dram_tensor(
                f"h{ell}_{cn}", [S_CLS, HIDDEN], f16, kind="Internal").ap()
    partials = [[nc.dram_tensor(f"partials{ell}_{d}", [NH, HIDDEN], f16,
                                kind="Internal").ap() for d in range(2)]
                for ell in range(DEPTH)]
    rs_mid = [[nc.dram_tensor(f"rsmid{ell}_{d}", [NH8, HIDDEN], f16,
                              kind="Internal").ap() for d in range(2)]
              for ell in range(DEPTH - 1)]
    tmp = [[nc.dram_tensor(f"tmp{ell}_{d}", [NH, HIDDEN], f16,
                           kind="Internal", addr_space="Shared").ap()
            for d in range(2)] for ell in range(DEPTH - 1)]
    rs_out = [nc.dram_tensor(f"rsout{d}", [NH8, HIDDEN], f16,
                             kind="Internal").ap() for d in range(2)]

    nc.gpsimd.load_library(library_config.mlp)

    with tile.TileContext(nc) as tc:
        with (
            tc.tile_pool(name="pers", bufs=1) as pers,
            tc.tile_pool(name="work", bufs=2) as work,
            tc.tile_pool(name="selp", bufs=1) as selp,
            tc.tile_pool(name="parp", bufs=1) as parp,
            tc.tile_pool(name="pse", bufs=2, space="PSUM") as pse,
            tc.tile_pool(name="psA", bufs=2, space="PSUM") as psA,
            tc.tile_pool(name="psT", bufs=4, space="PSUM") as psT,
        ):
            # ---------- persistent SBUF
            def pload(ap_in, shape, tag, dt=f16, eng="sync"):
                t = pers.tile(shape, dt, tag=tag)
                (nc.sync if eng == "sync" else nc.scalar).dma_start(t[:], ap_in)
                return t

            w_wia_a = pload(wia_a[:], [128, HIDDEN], "w_wia_a")
            w_wia_b = pload(wia_b[:], [ATOM_FDIM - 128, HIDDEN], "w_wia_b")
            w_wib = pload(wib[:], [BOND_FDIM, HIDDEN], "w_wib")
            w_wh = pload(wh_t[:], [HIDDEN, HIDDEN], "w_wh")
            w_woa_a = pload(woa_a[:], [128, HIDDEN], "w_woa_a")
            w_woa_b = pload(woa_b[:], [ATOM_FDIM - 128, HIDDEN], "w_woa_b")
            w_wom = pload(wom[:], [HIDDEN, HIDDEN], "w_wom")
            io_t = pload(iotaf[:], [128, WGRP * 128], "io_t")
            ident_s = pload(ident_t[:], [128, 128], "ident_s")
            sidx = pload(src16w[:], [128, S_TOT // 16], "sidx", i16)
            ridx = pload(rev16w[:], [128, S_TOT // 16], "ridx", i16,
                         eng="scalar")
            dstl_s = pload(dstl_t[:], [128, 4 * 2 * NWIN], "dstl_s",
                           eng="scalar")

            # ---------- redundant proj: full node table on every core
            # NH = 49*512, so 512-col chunks align to halves.
            for half in range(2):
                for ch in range(NH // 512):
                    base = half * NH + ch * 512
                    a_t = work.tile([128, 512], f16, tag="pa")
                    b_t = work.tile([ATOM_FDIM - 128, 512], f16, tag="pb")
                    nc.sync.dma_start(a_t[:], atomT_a[:, base:base + 512])
                    nc.scalar.dma_start(b_t[:], atomT_b[:, base:base + 512])
                    o_t = work.tile([128, 512], f16, tag="po")
                    for q in range(4):
                        ps = psA.tile([128, HIDDEN], f32, tag="sg")
                        qs = slice(q * 128, (q + 1) * 128)
                        nc.tensor.matmul(ps[:], lhsT=a_t[:, qs],
                                         rhs=w_wia_a[:], start=True,
                                         stop=False)
                        nc.tensor.matmul(ps[:], lhsT=b_t[:, qs],
                                         rhs=w_wia_b[:], start=False,
                                         stop=True)
                        nc.scalar.copy(o_t[:, qs], ps[:])
                    nc.sync.dma_start(
                        proj_d[half][ch * 512:(ch + 1) * 512, :].rearrange(
                            "(a p) d -> p a d", p=128),
                        o_t[:].rearrange("p (a d) -> p a d", d=HIDDEN))

            # ---------- layers with fused segment-sum
            for ell in range(DEPTH):
                for d in (0, 1):
                    par_t = parp.tile([128, NWIN * HIDDEN], f16, tag="par")
                    for ci_pos, cn in enumerate(CLS_OF_D[d]):
                        s = int(cn[0])
                        cls_idx = CLS_NAMES.index(cn)
                        o = cls_idx * S_CLS
                        rcn = REV_CLS[cn]
                        for g in range(NGRP):
                            nw = GRP_NW[g]
                            nsl = GRP_SL[g]
                            goff = int(GRP_OFF[g])
                            nov = ((nw + 7) // 8) * 128
                            w0 = g * WGRP
                            icols = slice((o + goff) // 16,
                                          (o + goff + nsl) // 16)
                            h_t = work.tile([128, GRP_SL[0]], f16, tag="ht")
                            if ell == 0:
                                g1 = work.tile([128, GRP_SL[0]], f16,
                                               tag="g1")
                                nc.gpsimd.dma_gather(
                                    g1[:, :nsl].rearrange(
                                        "p (c n) -> p c n", n=HIDDEN),
                                    proj_d[s][:], sidx[:, icols], nsl, nsl,
                                    HIDDEN, single_packet=False,
                                    queue_num=(2 * g + ci_pos) % 4)
                                bt = work.tile([BOND_FDIM, GRP_SL[0]], f16,
                                               tag="bt")
                                nc.scalar.dma_start(
                                    bt[:, :nsl],
                                    bondT[:, o + goff:o + goff + nsl])
                                for j in range((nsl + 511) // 512):
                                    nq = min(4, nsl // 128 - j * 4)
                                    ps = pse.tile([128, 512], f32, tag="mm")
                                    for q in range(nq):
                                        ci = j * 4 + q
                                        nc.tensor.matmul(
                                            ps[:, q * 128:(q + 1) * 128],
                                            lhsT=bt[:, ci * 128:
                                                    (ci + 1) * 128],
                                            rhs=w_wib[:], start=True,
                                            stop=True)
                                    sl = slice(j * 512, j * 512 + nq * 128)
                                    nc.vector.tensor_add(
                                        out=h_t[:, sl], in0=g1[:, sl],
                                        in1=ps[:, :nq * 128])
                                    nc.scalar.activation(h_t[:, sl],
                                                         h_t[:, sl], Relu)
                            else:
                                table = tmp[ell - 1][s]
                                g1 = work.tile([128, GRP_SL[0]], f16,
                                               tag="g1")
                                nc.gpsimd.dma_gather(
                                    g1[:, :nsl].rearrange(
                                        "p (c d) -> p c d", d=HIDDEN),
                                    table[:], sidx[:, icols], nsl, nsl,
                                    HIDDEN, single_packet=False,
                                    queue_num=(2 * g + ci_pos) % 4)
                                g2 = work.tile([128, GRP_SL[0]], f16,
                                               tag="g2")
                                nc.gpsimd.dma_gather(
                                    g2[:, :nsl].rearrange(
                                        "p (c d) -> p c d", d=HIDDEN),
                                    h_cls[(ell - 1, rcn)][:],
                                    ridx[:, icols], nsl, nsl,
                                    HIDDEN, single_packet=False,
                                    queue_num=(2 * g + ci_pos + 2) % 4)
                                nc.vector.tensor_tensor(
                                    out=g1[:, :nsl], in0=g1[:, :nsl],
                                    in1=g2[:, :nsl],
                                    op=mybir.AluOpType.subtract)
                                for j in range((nsl + 511) // 512):
                                    nq = min(4, nsl // 128 - j * 4)
                                    ps = pse.tile([128, 512], f32, tag="mm")
                                    mt = work.tile([128, 512], f16, tag="mt")
                                    tps = []
                                    for q in range(nq):
                                        ci = j * 4 + q
                                        if q % 2 == 0:
                                            tp = psT.tile([128, 256], f16,
                                                          tag="tp")
                                            tps.append(tp)
                                        nc.tensor.transpose(
                                            tps[-1][:, (q % 2) * 128:
                                                    (q % 2) * 128 + 128],
                                            g1[:, ci * 128:(ci + 1) * 128],
                                            ident_s[:])
                                    for t_i, tp in enumerate(tps):
                                        w_cols = min(256, nq * 128 -
                                                     t_i * 256)
                                        msl = slice(t_i * 256,
                                                    t_i * 256 + w_cols)
                                        if t_i % 2 == 0:
                                            nc.scalar.copy(mt[:, msl],
                                                           tp[:, :w_cols])
                                        else:
                                            nc.vector.tensor_copy(
                                                mt[:, msl], tp[:, :w_cols])
                                    for q in range(nq):
                                        msl = slice(q * 128, (q + 1) * 128)
                                        nc.tensor.matmul(
                                            ps[:, msl], lhsT=mt[:, msl],
                                            rhs=w_wh[:], start=True,
                                            stop=True)
                                    sl = slice(j * 512, j * 512 + nq * 128)
                                    nc.scalar.activation(
                                        h_t[:, sl], ps[:, :nq * 128], Relu)
                            if ell < DEPTH - 1:
                                nc.sync.dma_start(
                                    h_cls[(ell, cn)]
                                    [goff:goff + nsl, :].rearrange(
                                        "(c p) d -> p c d", p=128),
                                    h_t[:, :nsl].rearrange(
                                        "p (c d) -> p c d", d=HIDDEN))

                            # fused seg-sum for this group's windows
                            sel_d = selp.tile([128, WGRP * 128], f16,
                                              tag="seld")
                            sel_o = selp.tile([128, WGRP * 128], f16,
                                              tag="selo")
                            dc = cls_idx * 2 * NWIN + w0
                            sel_eng = nc.vector
                            sel_eng.tensor_tensor(
                                out=sel_d[:, :nw * 128].rearrange(
                                    "p (c n) -> p c n", n=128),
                                in0=io_t[:, :nw * 128].rearrange(
                                    "p (c n) -> p c n", n=128),
                                in1=dstl_s[:, dc:dc + nw]
                                .to_broadcast([128, nw, 128]),
                                op=mybir.AluOpType.is_equal)
                            dco = cls_idx * 2 * NWIN + NWIN + w0
                            sel_eng.tensor_tensor(
                                out=sel_o[:, :nw * 128].rearrange(
                                    "p (c n) -> p c n", n=128),
                                in0=io_t[:, :nw * 128].rearrange(
                                    "p (c n) -> p c n", n=128),
                                in1=dstl_s[:, dco:dco + nw]
                                .to_broadcast([128, nw, 128]),
                                op=mybir.AluOpType.is_equal)
                            ps2 = None
                            for wi in range(nw):
                                if wi % 2 == 0:
                                    ps2 = psA.tile([128, 2 * HIDDEN], f32,
                                                   tag="sg")
                                half = (wi % 2) * HIDDEN
                                nc.tensor.matmul(
                                    ps2[:, half:half + HIDDEN],
                                    lhsT=sel_d[:, wi * 128:(wi + 1) * 128],
                                    rhs=h_t[:, wi * HIDDEN:
                                            (wi + 1) * HIDDEN],
                                    start=True, stop=False)
                                oc = nw + wi // 8
                                nc.tensor.matmul(
                                    ps2[:, half:half + HIDDEN],
                                    lhsT=sel_o[:, wi * 128:(wi + 1) * 128],
                                    rhs=h_t[:, oc * HIDDEN:
                                            (oc + 1) * HIDDEN],
                                    start=False, stop=True)
                                if wi % 2 == 1:
                                    col = (w0 + wi - 1) * HIDDEN
                                    if ci_pos == 0:
                                        nc.scalar.copy(
                                            par_t[:, col:col + 2 * HIDDEN],
                                            ps2[:])
                                    else:
                                        nc.vector.tensor_add(
                                            out=par_t[:, col:
                                                      col + 2 * HIDDEN],
                                            in0=par_t[:, col:
                                                      col + 2 * HIDDEN],
                                            in1=ps2[:])
                            if ci_pos == 1:
                                nc.sync.dma_start(
                                    partials[ell][d]
                                    [w0 * 128:w0 * 128 + nw * 128, :]
                                    .rearrange("(a p) d -> p a d", p=128),
                                    par_t[:, w0 * HIDDEN:
                                          (w0 + nw) * HIDDEN].rearrange(
                                        "p (a d) -> p a d", d=HIDDEN))
                    # collectives per half
                    if debug_1core:
                        if ell < DEPTH - 1:
                            nc.sync.dma_start(tmp[ell][d][:],
                                              partials[ell][d][:])
                        else:
                            nc.sync.dma_start(rs_out[d][:],
                                              partials[ell][d][0:NH8, :])
                    elif ell < DEPTH - 1:
                        if use_ar:
                            nc.gpsimd.collective_compute(
                                "AllReduce", mybir.AluOpType.add,
                                replica_groups=[list(range(N_CORES))],
                                ins=[partials[ell][d][:]],
                                outs=[tmp[ell][d][:]])
                        else:
                            nc.gpsimd.collective_compute(
                                "ReduceScatter", mybir.AluOpType.add,
                                replica_groups=[list(range(N_CORES))],
                                ins=[partials[ell][d][:]],
                                outs=[rs_mid[ell][d][:]])
                            nc.gpsimd.collective_compute(
                                "AllGather", mybir.AluOpType.bypass,
                                replica_groups=[list(range(N_CORES))],
                                ins=[rs_mid[ell][d][:]],
                                outs=[tmp[ell][d][:]])
                    else:
                        nc.gpsimd.collective_compute(
                            "ReduceScatter", mybir.AluOpType.add,
                            replica_groups=[list(range(N_CORES))],
                            ins=[partials[ell][d][:]], outs=[rs_out[d][:]])

            if dbg_dump:
                nc.sync.dma_start(dbg["tmp1_0"][:], tmp[1][0][:])
                nc.scalar.dma_start(dbg["par1_0"][:], partials[1][0][:])
                nc.sync.dma_start(dbg["par2_0"][:], partials[2][0][:])
                nc.scalar.dma_start(dbg["rso_0"][:], rs_out[0][:])
                nc.sync.dma_start(dbg["h1_00"][:], h_cls[(1, "00")][:])

            # ---------- final: out.T = relu(WoA@atom.T + WoM@msg.T)
            for ch in range(OUT_COLS // 128):
                csl = slice(ch * 128, (ch + 1) * 128)
                a_t = work.tile([128, 128], f16, tag="fa")
                b_t = work.tile([ATOM_FDIM - 128, 128], f16, tag="fb")
                m_t = work.tile([128, 128], f16, tag="fm")
                mraw = work.tile([128, 128], f16, tag="fmr")
                nc.sync.dma_start(a_t[:], atomP_a[:, csl])
                nc.scalar.dma_start(b_t[:], atomP_b[:, csl])
                r0 = ch * 128
                r1 = (ch + 1) * 128
                if r1 <= NH8:
                    nc.sync.dma_start(mraw[:], rs_out[0][r0:r1, :])
                elif r0 >= NH8:
                    nc.sync.dma_start(mraw[:], rs_out[1][r0 - NH8:r1 - NH8, :])
                else:
                    k = NH8 - r0
                    nc.sync.dma_start(mraw[:k, :], rs_out[0][r0:NH8, :])
                    nc.sync.dma_start(mraw[k:, :], rs_out[1][0:r1 - NH8, :])
                tpf = psT.tile([128, 128], f16, tag="tp")
                nc.tensor.transpose(tpf[:], mraw[:], ident_s[:])
                nc.scalar.copy(m_t[:], tpf[:])
                ps = psA.tile([128, 128], f32, tag="sg")
                nc.tensor.matmul(ps[:], lhsT=w_woa_a[:], rhs=a_t[:],
                                 start=True, stop=False)
                nc.tensor.matmul(ps[:], lhsT=w_woa_b[:], rhs=b_t[:],
                                 start=False, stop=False)
                nc.tensor.matmul(ps[:], lhsT=w_wom[:], rhs=m_t[:],
                                 start=False, stop=True)
                o_t = work.tile([128, 128], f32, tag="fob")
                nc.scalar.activation(o_t[:], ps[:], Relu)
                nc.sync.dma_start(out_t[:, csl], o_t[:])

    # Tile assigns SWDGE completion sems round-robin (DMASW<i>_*); the HW
    # locks each sem to one SWDGE queue, so spread gathers across the 4
    # queues by their assigned sem index.
    import re
    for b in nc.main_func.blocks:
        for ins in b.instructions:
            if type(ins).__name__ == "InstDMAGatherAnt" and ins.sync_info:
                for upd in ins.sync_info.on_update:
                    mname = upd.ant_name or ""
                    mm = re.match(r"DMASW(\d+)_", mname)
                    if mm:
                        ins.queue_num = int(mm.group(1)) % 4
                        break

    nc.compile()
    return nc


# -------------------------------------------------------------------- entry
_CACHE = {}


def kernel(atom_feats, bond_feats, W_i, W_h, W_o, src, dst, reverse_e):
    from concourse import bass_utils

    rev = np.asarray(reverse_e).astype(np.int64)
    ar = np.arange(N_PAIRS, dtype=np.int64)
    assert np.array_equal(rev[:N_PAIRS], ar + N_PAIRS) and \
        np.array_equal(rev[N_PAIRS:], ar), "unexpected reverse_e structure"

    meta, in_maps = _host_prep(atom_feats, bond_feats, W_i, W_h, W_o, src, dst)

    key = (meta["S_CLS"],)
    if key not in _CACHE:
        _CACHE[key] = _build_program(meta)
    nc = _CACHE[key]

    res = bass_utils.run_bass_kernel_spmd(
        nc, in_maps, core_ids=list(range(N_CORES)))
    out = np.empty((NPAD, HIDDEN), dtype=np.float32)
    for c in range(N_CORES):
        o = res.results[c]["out"].T.astype(np.float32)   # [OUT_COLS, H]
        out[c * NH8:(c + 1) * NH8] = o[:NH8]
        out[NH + c * NH8:NH + (c + 1) * NH8] = o[NH8:]
    return np.ascontiguousarray(out[:N_NODES])
